# revision 3
# baseline (speedup 1.0000x reference)
"""GroupedTernaryLinear Trainium2 kernel v3 (Bass/Tile, 8-core SPMD).

Computation (matches the jax reference):
  x:      [2, 4096, 4096] f32   -> flatten to [8192, 4096] tokens
  weight: [4096, 1024]    f32
  1. xn = rms_norm(x) over last dim (eps = f32 eps)
  2. w_bf = bf16(weight); per flat 64-chunk: scale = bf16(mean|w_bf|) (clipped),
     q = clip(round(w_bf/scale), -1, 1)  ->  wq = q*scale  (exact in bf16)
  3. out[t, g*1024+o] = sum_i xn[t, g*1024+i] * wq[g*1024+o, i]   (4 groups)

Sharding: 2 token-halves x 4 groups = 8 cores. Core c = 4*i + j gets
tokens [4096*i, 4096*(i+1)) and group j. The rms-norm's full-feature sum
of squares is AllReduced across each token half's 4 group-shards in two
16-block chunks; the factor is folded into output evacuation.

v3 design notes (from v1=219us / v2=314us traces):
  - PE runs the 512 main matmuls + the 64 weight transposes (which fall
    in the PE's otherwise-idle startup window and warm the HAM clock
    gate). x transposes go through the DMA XBAR from an explicit bf16
    cast tile.
  - The weight quantization is a ~6us/tile serial chain on the vector
    engine (the broadcast-view compare/scale ops run ~1.2us each) and is
    the startup critical path: wqT_lo ~33us, wqT_hi ~57us. So the first
    H0S=13 blocks run lo-half-only matmuls (2.1us each) bridging exactly
    to wqT_hi readiness, then an h1 catch-up pass, then the interleaved
    lo+hi main loop. PE never idles once started.
  - Uniform output flow: every block's output is scaled+written as soon
    as its factor exists; earlier blocks stage raw bf16 and are flushed
    2-per-iteration once fac lands. Direct-from-PSUM from block 25.
  - Queue plan (steady state): sync = x-in DMA + x XBAR transpose;
    scalar = squares + staged-lo scales + 1/3 out DMA; vector = quant,
    then evacuation (direct scales / staged-hi) ; gpsimd = collectives +
    1/3 out DMA. The x bf16 cast rotates vector/scalar/gpsimd early
    (all three have slack mid-startup), vector-only in steady state.
  - Tiny warm-up ops at t=0 pre-load the DVE opcode tables and the ACT
    tables (Square/Copy/Sqrt) so first-use ~1.3-3us hits don't land on
    the critical path.
"""

import os
import sys

sys.path.insert(0, "/opt/trn_rl_repo")

import numpy as np

import concourse.bass as bass
import concourse.mybir as mybir
import concourse.tile as tile
from concourse import bacc
from concourse.bass_utils import run_bass_kernel_spmd
from concourse.masks import make_identity

F32 = mybir.dt.float32
BF16 = mybir.dt.bfloat16
AF = mybir.ActivationFunctionType
ALU = mybir.AluOpType

N_CORES = 8
TOK = 4096        # tokens per core
DIN = 1024        # per-core input features (one group)
DOUT = 1024       # per-core outputs (one group)
DFULL = 4096      # full feature dim (norm denominator)
TB = TOK // 128   # 32 token blocks
GK = DIN // 128   # 8 k-chunks of 128
EPS = 1.1920929e-07          # np.finfo(np.float32).eps
THR = 0.5009765625           # bf16 round-to-nearest-even threshold for |r|>0.5

H0S = 13          # lo-only blocks while w tiles 4-7 quantize
DIRECT_FROM = 25  # blocks from here scale straight out of PSUM
CC_SPLIT = 16     # ss AllReduce chunk boundary
FAC_A_ITER = 15   # main-loop iter: fac_a readback + chunk-A flushes start
FAC_B_ITER = 22   # same for chunk B

LAST_EXEC_NS = None
LAST_RESULTS = None


def _build():
    nc = bacc.Bacc("TRN2", target_bir_lowering=False, debug=False, num_devices=8)
    x_ap = nc.dram_tensor("x", [TOK, DIN], F32, kind="ExternalInput").ap()
    w_ap = nc.dram_tensor("weight", [DOUT, DIN], F32, kind="ExternalInput").ap()
    out_ap = nc.dram_tensor("out", [TOK, DOUT], F32, kind="ExternalOutput").ap()

    with tile.TileContext(nc) as tc:
        _body(tc, nc, out_ap, x_ap, w_ap)

    nc.compile()
    return nc


def _body(tc, nc, out_ap, x_ap, w_ap):
    with (
        tc.tile_pool(name="consts", bufs=1) as consts,
        tc.tile_pool(name="warm", bufs=1) as warm_pool,
        tc.tile_pool(name="wqt", bufs=1) as wqt_pool,
        tc.tile_pool(name="rawp", bufs=1) as raw_pool,
        tc.tile_pool(name="win", bufs=6) as win_pool,
        tc.tile_pool(name="wtmp", bufs=2) as wtmp_pool,
        tc.tile_pool(name="wst", bufs=2) as wst_pool,
        tc.tile_pool(name="xin", bufs=5) as xin_pool,
        tc.tile_pool(name="xbp", bufs=4) as xb_pool,
        tc.tile_pool(name="xta", bufs=16) as xta_pool,
        tc.tile_pool(name="stats", bufs=1) as stats_pool,
        tc.tile_pool(name="obp", bufs=5) as ob_pool,
        tc.tile_pool(name="dram", bufs=1, space="DRAM") as dram_pool,
        tc.tile_pool(name="ps_mm", bufs=6, space="PSUM") as ps_mm,
        tc.tile_pool(name="ps_wtp", bufs=2, space="PSUM") as ps_wtp,
    ):
        eps_t = consts.tile([128, 1], F32, name="eps_t")
        nc.vector.memset(eps_t[:], EPS)
        ident_b = consts.tile([128, 128], BF16, name="ident_b")
        make_identity(nc, ident_b[:])

        # ---- engine warm-ups: pre-load opcode/ACT tables -----------------
        wa = warm_pool.tile([128, 4], F32, name="wa")
        wb = warm_pool.tile([128, 4], F32, name="wb")
        wc = warm_pool.tile([128, 4], BF16, name="wc")
        wd = warm_pool.tile([128, 1], F32, name="wd")
        nc.vector.memset(wa[:], 1.0)
        nc.vector.memset(wb[:], 2.0)
        nc.vector.tensor_tensor(wc[:], wa[:], wb[:], ALU.is_gt)
        nc.vector.tensor_scalar(wb[:], wa[:], 0.5, 1e-8, ALU.mult, ALU.max)
        nc.vector.tensor_reduce(wd[:], wa[:], axis=mybir.AxisListType.X, op=ALU.add,
                                apply_absolute_value=True)
        nc.vector.tensor_copy(wc[:], wa[:])
        nc.vector.reciprocal(wd[:], wd[:])
        nc.scalar.activation(wc[:], wa[:], AF.Square, accum_out=wd[:])
        nc.scalar.activation(wb[:], wa[:], AF.Copy, scale=0.5)
        nc.scalar.activation(wd[:], wd[:], AF.Sqrt, bias=eps_t[:], scale=0.5)
        nc.gpsimd.tensor_copy(wc[:], wa[:])

        # Quantized transposed weight: [i(128), k, o-half]
        wqT_lo = wqt_pool.tile([128, GK, 512], BF16, name="wqT_lo")
        wqT_hi = wqt_pool.tile([128, GK, 512], BF16, name="wqT_hi")

        ss_all = stats_pool.tile([128, TB], F32, name="ss_all")
        ss_sum_a = stats_pool.tile([128, CC_SPLIT], F32, name="ss_sum_a")
        ss_sum_b = stats_pool.tile([128, TB - CC_SPLIT], F32, name="ss_sum_b")
        sq_a = stats_pool.tile([128, CC_SPLIT], F32, name="sq_a")
        sq_b = stats_pool.tile([128, TB - CC_SPLIT], F32, name="sq_b")
        fac_a = stats_pool.tile([128, CC_SPLIT], F32, name="fac_a")
        fac_b = stats_pool.tile([128, TB - CC_SPLIT], F32, name="fac_b")
        junk = stats_pool.tile([128, DIN], BF16, name="junk")

        cc_in_a = dram_pool.tile([128, CC_SPLIT], F32, name="cc_in_a")
        cc_out_a = dram_pool.tile([128, CC_SPLIT], F32, name="cc_out_a")
        cc_in_b = dram_pool.tile([128, TB - CC_SPLIT], F32, name="cc_in_b")
        cc_out_b = dram_pool.tile([128, TB - CC_SPLIT], F32, name="cc_out_b")

        def fac_ap(b):
            if b < CC_SPLIT:
                return fac_a[:, b:b + 1]
            return fac_b[:, b - CC_SPLIT:b - CC_SPLIT + 1]

        w_tiles = {}
        wq_tiles = {}
        x_pending = []   # (b, f32 tile) in DMA flight
        xb_tiles = {}    # b -> bf16 cast tile
        xT = {}          # b -> [128, GK, 128] bf16 transposed tile
        staged = {}      # b -> ("full", tile, None) | ("halves", lo, hi)
        pm_live = {}     # b -> (pm_lo, pm_hi) awaiting evacuation

        def emit_wdma(t):
            w_t = win_pool.tile([128, DIN], F32, name="w_t")
            nc.scalar.dma_start(w_t[:], w_ap[t * 128:(t + 1) * 128, :])
            w_tiles[t] = w_t

        def emit_wbf(t):
            w_t = w_tiles.pop(t)
            wbf = wtmp_pool.tile([128, DIN], BF16, name="wbf", bufs=4)
            nc.scalar.copy(wbf[:], w_t[:])              # f32 -> bf16 (RNE)
            wq_tiles[t] = wbf

        def emit_wquant(t):
            wbf = wq_tiles.pop(t)
            wbf_v = wbf[:].rearrange("p (c q) -> p c q", q=64)

            red = wst_pool.tile([128, 16], F32, name="red")
            nc.vector.tensor_reduce(
                red[:], wbf_v, axis=mybir.AxisListType.X, op=ALU.add,
                apply_absolute_value=True,
            )
            s_bf = wst_pool.tile([128, 16], BF16, name="s_bf")
            nc.vector.tensor_scalar(
                s_bf[:], red[:], 1.0 / 64.0, 1e-8, ALU.mult, ALU.max,
            )
            thr_p = wst_pool.tile([128, 16], F32, name="thr_p")
            nc.vector.tensor_scalar_mul(thr_p[:], s_bf[:], THR)
            thr_n = wst_pool.tile([128, 16], F32, name="thr_n")
            nc.vector.tensor_scalar_mul(thr_n[:], s_bf[:], -THR)

            # q = (w > t) - (w < -t); wq = q*s  (broadcast views)
            tp_b = thr_p[:].unsqueeze(2).broadcast_to((128, 16, 64))
            tn_b = thr_n[:].unsqueeze(2).broadcast_to((128, 16, 64))
            s_b = s_bf[:].unsqueeze(2).broadcast_to((128, 16, 64))
            mp = wtmp_pool.tile([128, DIN], BF16, name="mp")
            mp_v = mp[:].rearrange("p (c q) -> p c q", q=64)
            nc.vector.tensor_tensor(mp_v, wbf_v, tp_b, ALU.is_gt)
            mn = wtmp_pool.tile([128, DIN], BF16, name="mn")
            mn_v = mn[:].rearrange("p (c q) -> p c q", q=64)
            nc.vector.tensor_tensor(mn_v, wbf_v, tn_b, ALU.is_lt)
            nc.vector.tensor_sub(mp[:], mp[:], mn[:])
            wqv = wtmp_pool.tile([128, DIN], BF16, name="wqv")
            wqv_v = wqv[:].rearrange("p (c q) -> p c q", q=64)
            nc.vector.tensor_tensor(wqv_v, mp_v, s_b, ALU.mult)

            # PE transpose (the PE is idle in this window + this warms HAM):
            # wqT[i, k, (t%4)*128+o] = wqv[o, k*128+i]
            dst = wqT_lo if t < 4 else wqT_hi
            off = (t % 4) * 128
            wps = ps_wtp.tile([128, GK, 128], BF16, name="wps")
            for k in range(GK):
                nc.tensor.transpose(
                    wps[:, k, :], wqv[:, k * 128:(k + 1) * 128], ident_b[:],
                )
            nc.scalar.copy(dst[:, :, off:off + 128], wps[:])

        def emit_xdma(b):
            x_t = xin_pool.tile([128, DIN], F32, name="x_t")
            nc.sync.dma_start(x_t[:], x_ap[b * 128:(b + 1) * 128, :])
            x_pending.append((b, x_t))

        def emit_xcast(b, eng):
            bb, x_t = x_pending.pop(0)
            assert bb == b, (bb, b)
            nc.scalar.activation(
                junk[:], x_t[:], AF.Square, accum_out=ss_all[:, b:b + 1],
            )
            xb = xb_pool.tile([128, DIN], BF16, name="xb")
            if eng is nc.scalar:
                nc.scalar.copy(xb[:], x_t[:])
            else:
                eng.tensor_copy(xb[:], x_t[:])
            xb_tiles[b] = xb

        def emit_xtr(b):
            xb = xb_tiles.pop(b)
            xt = xta_pool.tile([128, GK, 128], BF16, name="xt")
            nc.sync.dma_start(xt[:], xb[:], transpose=True)
            xT[b] = xt

        def emit_chain(b):
            if b + 5 < TB:
                emit_xdma(b + 5)
            eng = (nc.vector, nc.scalar, nc.gpsimd)[b % 3] if b < 26 else nc.vector
            emit_xcast(b, eng)
            emit_xtr(b)

        def emit_mm8(b, half):
            w = wqT_lo if half == 0 else wqT_hi
            pm = ps_mm.tile([128, 512], F32, name="pm")
            for k in range(GK):
                nc.tensor.matmul(
                    pm[:], xT[b][:, k, :], w[:, k, :],
                    start=(k == 0), stop=(k == GK - 1),
                )
            if half == 0:
                # staged-lo scale-less evacuation on scalar (vector is busy
                # quantizing tiles 4-7 in this window)
                rhh = raw_pool.tile([128, 512], BF16, name=f"rh{b}_0")
                nc.scalar.activation(rhh[:], pm[:], AF.Copy)
                staged[b] = ["halves", rhh, None]
            else:
                rhh = raw_pool.tile([128, 512], BF16, name=f"rh{b}_1")
                nc.vector.tensor_copy(rhh[:], pm[:])
                staged[b][2] = rhh

        def emit_mm16(b):
            pml = ps_mm.tile([128, 512], F32, name="pm")
            pmh = ps_mm.tile([128, 512], F32, name="pm")
            for k in range(GK):
                nc.tensor.matmul(
                    pml[:], xT[b][:, k, :], wqT_lo[:, k, :],
                    start=(k == 0), stop=(k == GK - 1),
                )
                nc.tensor.matmul(
                    pmh[:], xT[b][:, k, :], wqT_hi[:, k, :],
                    start=(k == 0), stop=(k == GK - 1),
                )
            pm_live[b] = (pml, pmh)

        def out_dma(b, ob):
            oeng = (nc.gpsimd, nc.sync, nc.scalar)[b % 3]
            oeng.dma_start(out_ap[b * 128:(b + 1) * 128, :], ob[:])

        def emit_evac(b):
            pml, pmh = pm_live.pop(b)
            if b >= DIRECT_FROM:
                ob = ob_pool.tile([128, DOUT], F32, name="ob")
                nc.vector.tensor_scalar_mul(ob[:, 0:512], pml[:], fac_ap(b))
                nc.vector.tensor_scalar_mul(ob[:, 512:1024], pmh[:], fac_ap(b))
                out_dma(b, ob)
            else:
                rhb = raw_pool.tile([128, DOUT], BF16, name=f"rh{b}")
                nc.scalar.activation(rhb[:, 0:512], pml[:], AF.Copy)
                nc.vector.tensor_copy(rhb[:, 512:1024], pmh[:])
                staged[b] = ["full", rhb, None]

        def emit_flush(b):
            ent = staged.pop(b)
            ob = ob_pool.tile([128, DOUT], F32, name="ob")
            if ent[0] == "full":
                rhb = ent[1]
                lo, hi = rhb[:, 0:512], rhb[:, 512:1024]
            else:
                lo, hi = ent[1][:], ent[2][:]
            nc.scalar.activation(ob[:, 0:512], lo, AF.Copy, scale=fac_ap(b))
            nc.vector.tensor_scalar_mul(ob[:, 512:1024], hi, fac_ap(b))
            out_dma(b, ob)

        def emit_cc(chunk):
            if chunk == 0:
                cc_in, cc_out, sl = cc_in_a, cc_out_a, slice(0, CC_SPLIT)
            else:
                cc_in, cc_out, sl = cc_in_b, cc_out_b, slice(CC_SPLIT, TB)
            nc.gpsimd.dma_start(cc_in[:], ss_all[:, sl])
            nc.gpsimd.collective_compute(
                "AllReduce",
                ALU.add,
                replica_groups=[[0, 1, 2, 3], [4, 5, 6, 7]],
                ins=[cc_in.opt()],
                outs=[cc_out.opt()],
            )

        def emit_fac(chunk):
            if chunk == 0:
                cc_out, ss_sum, sq, fac = cc_out_a, ss_sum_a, sq_a, fac_a
            else:
                cc_out, ss_sum, sq, fac = cc_out_b, ss_sum_b, sq_b, fac_b
            nc.gpsimd.dma_start(ss_sum[:], cc_out[:])
            nc.scalar.activation(
                sq[:], ss_sum[:], AF.Sqrt, bias=eps_t[:], scale=1.0 / DFULL,
            )
            nc.vector.reciprocal(fac[:], sq[:])

        # ---- emission ------------------------------------------------------
        for t in range(4):
            emit_wdma(t)
        for b in range(5):
            emit_xdma(b)
        for t in range(4):
            emit_wbf(t)
        # pre-chain the first three x blocks so their bf16 casts run on the
        # vector queue before the quant chain monopolizes it
        for b in range(3):
            emit_xcast(b, nc.vector)
            emit_xtr(b)
        for t in range(4):
            emit_wquant(t)
            emit_wdma(4 + t)
            emit_wbf(4 + t)

        # h0 phase: blocks 0..H0S-1 on the lo half; w tiles 4-7 quantize on
        # vector interleaved (one tile every 3rd iteration).
        for b in range(H0S):
            if b >= 3:
                emit_chain(b)
            elif b + 5 < TB:
                emit_xdma(b + 5)
            if b in (1, 4, 7, 10):
                emit_wquant(4 + (b - 1) // 3)
            emit_mm8(b, 0)

        # h1 catch-up; keep the x pipeline moving.
        for j in range(H0S):
            emit_chain(H0S + j)
            if H0S + j + 1 == CC_SPLIT:
                emit_cc(0)
            emit_mm8(j, 1)

        # main loop
        flushq = list(range(DIRECT_FROM))
        next_chain = 2 * H0S
        for b in range(H0S, TB):
            if next_chain < TB:
                emit_chain(next_chain)
                next_chain += 1
                if next_chain == CC_SPLIT:
                    emit_cc(0)
                elif next_chain == TB:
                    emit_cc(1)
            emit_mm16(b)
            if b - 1 in pm_live:
                emit_evac(b - 1)
            if b == FAC_A_ITER:
                emit_fac(0)
            if b == FAC_B_ITER:
                emit_fac(1)
            nflush = 0
            while nflush < 2 and flushq:
                fb = flushq[0]
                ok = (b >= FAC_A_ITER) if fb < CC_SPLIT else (b >= FAC_B_ITER)
                if not ok or fb not in staged:
                    break
                emit_flush(flushq.pop(0))
                nflush += 1
        # drain
        for b in sorted(pm_live):
            emit_evac(b)
        for fb in flushq:
            if fb in staged:
                emit_flush(fb)


_NC_CACHE = None


def _ensure_ntff_hook():
    """Install the antenv.axon_hooks shim + ctypes NTFF hook if missing."""
    import types

    try:
        from antenv.axon_hooks import get_axon_ntff_profile_hook  # noqa: F401
        return
    except ImportError:
        pass
    import antenv

    mod = types.ModuleType("antenv.axon_hooks")
    mod._hook = None
    mod.set_axon_ntff_profile_hook = lambda h: setattr(mod, "_hook", h)
    mod.get_axon_ntff_profile_hook = lambda: mod._hook
    sys.modules["antenv.axon_hooks"] = mod
    antenv.axon_hooks = mod
    try:
        if "/root/.axon_site" not in sys.path:
            sys.path.insert(0, "/root/.axon_site")
        from trn_agent_boot.trn_boot import _ntff_profile_via_ctypes

        mod.set_axon_ntff_profile_hook(
            _ntff_profile_via_ctypes("/opt/axon/libaxon_pjrt.so")
        )
    except Exception:
        pass


def kernel(x: np.ndarray, weight: np.ndarray) -> np.ndarray:
    global LAST_EXEC_NS, LAST_RESULTS, _NC_CACHE
    x = np.ascontiguousarray(np.asarray(x, dtype=np.float32))
    weight = np.ascontiguousarray(np.asarray(weight, dtype=np.float32))
    lead = x.shape[:-1]
    xf = x.reshape(-1, DFULL)
    assert xf.shape[0] == 2 * TOK, xf.shape

    if _NC_CACHE is None:
        _NC_CACHE = _build()
    nc = _NC_CACHE

    in_maps = []
    for i in range(2):
        for j in range(4):
            in_maps.append({
                "x": np.ascontiguousarray(
                    xf[i * TOK:(i + 1) * TOK, j * DIN:(j + 1) * DIN]
                ),
                "weight": np.ascontiguousarray(
                    weight[j * DOUT:(j + 1) * DOUT, :]
                ),
            })
    trace = bool(int(os.environ.get("CCK_TRACE", "0")))
    kw = {}
    if trace:
        _ensure_ntff_hook()
        tdir = os.environ.get("CCK_TRACE_DIR")
        if tdir:
            os.makedirs(tdir, exist_ok=True)
            kw["tmpdir"] = tdir
    res = run_bass_kernel_spmd(nc, in_maps, list(range(N_CORES)), trace=trace, **kw)
    LAST_EXEC_NS = res.exec_time_ns
    LAST_RESULTS = res
    out = np.empty((2 * TOK, DFULL), dtype=np.float32)
    for i in range(2):
        for j in range(4):
            out[i * TOK:(i + 1) * TOK, j * DOUT:(j + 1) * DOUT] = (
                res.results[i * 4 + j]["out"]
            )
    return out.reshape(*lead, DFULL)


if __name__ == "__main__":
    rng = np.random.default_rng(0)
    x = rng.standard_normal((2, 4096, 4096), dtype=np.float32)
    w = (rng.standard_normal((4096, 1024), dtype=np.float32) * 0.02).astype(np.float32)
    o = kernel(x, w)
    print(o.shape, o.dtype, LAST_EXEC_NS)
